# revision 55
# baseline (speedup 1.0000x reference)
"""Expert-parallel MoE kernel for one TRN2 chip (8 NeuronCores).

Strategy (expert-parallel, top-2 sparse):
  - core e owns expert e's weights. Weights arrive as bf16 copies of
    w{1,2}_v; they are loaded into SBUF pre-transposed via HWDGE
    DMA-transpose. Weight-norm is never applied to the big tensors:
    GEMM1 folds the per-row scale s1=g1/||v1|| into the Silu
    activation's per-partition `scale`, GEMM2 folds s2=g2/||v2|| into
    w2T once (per-column DVE multiply). The scales themselves are
    computed on device from the bf16 weights (square + ones-matmul
    partition reduction).
  - gating is data-parallel and runs FIRST: core i computes top-2
    masks for its T/8 token slice with exact fp32 matmuls (so routing
    never flips), then an AllToAll exchanges per-expert mask rows so
    every core holds the full-T mask for ITS expert. Weight DMA
    overlaps with gating/positions on the DMA engines.
  - token compaction: matmul prefix-sums turn the mask into global
    compacted positions; the gather list is built with windowed
    one-hot matmuls (windows from host-side routing, margin 64).
  - x arrives as a zero-padded bf16 copy; expert FFN gathers its
    compacted tokens transposed via dma_gather, runs dense GEMMs
    (bf16 compute / fp32 accumulate), and scatter-adds outputs back
    to token order (dropped tokens land in dump rows).
  - one ReduceScatter sums the 8 expert partials; each core emits its
    T/8-row shard via a casting DRAM->DRAM DMA.
"""

import numpy as np

import concourse.bass as bass
import concourse.mybir as mybir
import concourse.tile as tile
from concourse import bacc
from concourse.library_config import mlp

F32 = mybir.dt.float32
BF16 = mybir.dt.bfloat16
I16 = mybir.dt.int16

AX = mybir.AxisListType
OP = mybir.AluOpType
ACT = mybir.ActivationFunctionType


class Cfg:
    def __init__(self, T=8192, D=1024, H=4096, E=8, NCORES=8, C=2304,
                 use_silu=True):
        self.T, self.D, self.H, self.E = T, D, H, E
        self.NCORES = NCORES
        self.C = C               # per-expert token capacity (multiple of 128)
        self.TSL = T // NCORES   # tokens per core slice
        self.NCH = T // 128      # 128-token chunks
        self.DUMP = 64           # spread dump rows
        self.NRS = 8             # ReduceScatter chunks
        self.use_silu = use_silu
        # FFN token groups: 256-wide, with a 128 tail when C allows
        self.groups = []
        off = 0
        while off < C:
            s = 256 if C - off >= 256 else 128
            self.groups.append((off, s))
            off += s
        assert C % 128 == 0
        assert T % (NCORES * 128) == 0 and D % 128 == 0 and H % 128 == 0
        assert self.NCH <= 128


def build_moe(nc, cfg: Cfg, windows=None, debug=False):
    T, D, H, E, C = cfg.T, cfg.D, cfg.H, cfg.E, cfg.C
    TSL, NCH, DUMP = cfg.TSL, cfg.NCH, cfg.DUMP
    ND = D // 128   # D chunks of 128
    NH = H // 128   # H chunks of 128
    NDN = max(1, D // 512)       # GEMM2 output-column tiles
    DN = min(512, D)
    NCORES = cfg.NCORES
    NBJ = C // 128               # position blocks for gather-list build
    if windows is None:
        windows = [(J, NCH - 1) for J in range(NBJ)]
    # per-FFN-group token-row range touched by its scatter (from windows)
    grange = []
    for goff, S in cfg.groups:
        js = range(goff // 128, (goff + S) // 128)
        r0 = min(windows[J][0] for J in js) * 128
        r1 = (max(windows[J][1] for J in js) + 1) * 128
        grange.append((r0, r1))
    NRS = cfg.NRS                 # ReduceScatter chunks
    RSC = T // NRS

    # ---------------- kernel I/O ----------------
    xbfp = nc.dram_tensor("xbfp", [T + DUMP, D], BF16, kind="ExternalInput").ap()
    xslice = nc.dram_tensor("xslice", [TSL, D], F32, kind="ExternalInput").ap()
    gatev = nc.dram_tensor("gatev", [E, D], F32, kind="ExternalInput").ap()
    gateg = nc.dram_tensor("gateg", [E, 1], F32, kind="ExternalInput").ap()
    w1b = nc.dram_tensor("w1b", [H, D], BF16, kind="ExternalInput").ap()
    w2b = nc.dram_tensor("w2b", [D, H], BF16, kind="ExternalInput").ap()
    NCC = 721 + 2 * D
    cblk = nc.dram_tensor("cblk", [128, NCC], F32, kind="ExternalInput").ap()
    ones1b = nc.dram_tensor("ones1b", [1, 128], BF16, kind="ExternalInput").ap()
    onescb = nc.dram_tensor("onescb", [128, 1], BF16, kind="ExternalInput").ap()
    out_ext = nc.dram_tensor("out", [TSL, D], F32, kind="ExternalOutput").ap()

    # ---------------- internal DRAM ----------------
    a2a_in = nc.dram_tensor("a2a_in", [E * TSL], F32).ap()
    a2a_out = nc.dram_tensor("a2a_out", [E * TSL], F32).ap()
    rs_in = nc.dram_tensor("rs_in", [T + DUMP, D], BF16).ap()
    rs_out = nc.dram_tensor("rs_out", [TSL, D], BF16).ap()

    rg = [list(range(NCORES))]

    with tile.TileContext(nc) as tc:
        nc.gpsimd.load_library(mlp)

        # Pool layout: cpool/wpool/spool stay open for the whole kernel
        # (two parallel arenas); prep -> mid -> gemm are sequential scopes
        # reusing one arena in their natural time order. Scope boundaries
        # serialize (a later pool's first tile waits for every release of
        # the earlier one), so only naturally-ordered phases share space.
        with (
            tc.tile_pool(name="consts", bufs=1) as cpool,
            tc.tile_pool(name="wbig", bufs=1) as wpool,
            tc.tile_pool(name="scl", bufs=2) as spool,
            tc.tile_pool(name="psS", bufs=1, space="PSUM") as psps,
        ):
            # ---- constants: one packed DMA (tiny DMAs serialize at ~2.5us
            # completion latency each and stall the gating chain) ----
            cb = cpool.tile([128, NCC], F32)
            nc.sync.dma_start(cb[:], cblk)
            u128_sb = cb[:, 0:128]
            id_sb = cb[:, 128:256]
            jg_sb = cb[:, 256:384]
            tv_sb = cb[:, 384:384 + NCH]
            dumpo_sb = cb[:, 448:449]
            g1w_sb = cb[:, 449:449 + NH]
            b1w_sb = cb[:, 481:481 + NH]
            g2r_sb = cb[0:1, 721 + D:721 + 2 * D]
            uN_sb = cb[0:NCH, 521:521 + NCH]
            ones1_sb = cb[0:1, 585:713]
            gateb_sb = cb[0:1, 713:713 + E]
            b2r_sb = cb[0:1, 721:721 + D]
            ones1b_sb = cpool.tile([1, 128], BF16)
            nc.sync.dma_start(ones1b_sb[:], ones1b)
            onescb_sb = cpool.tile([128, 1], BF16)
            nc.sync.dma_start(onescb_sb[:], onescb)
            gwT = cpool.tile([128, ND, E], F32)       # normalized gate W^T
            s1w = cpool.tile([128, NH], F32)          # GEMM1 activation scale
            s2t = cpool.tile([128, D], F32)           # GEMM2 per-column scale
            b2pb = cpool.tile([1, D], BF16)           # b2 / s2, bf16
            glw = cpool.tile([128, 2 * (C // 16)], I16)  # gather+scatter lists
            w1T = wpool.tile([128, ND, H], BF16)      # [d, dc, h] = w1_v^T (bf16)
            w2T = wpool.tile([128, NH, D], BF16)      # [h, hc, d] = w2_v^T (bf16)

            # ================ gating (exact f32) -> maskT, A2A ================
            with (
                tc.tile_pool(name="prep", bufs=2) as ppool,
                tc.tile_pool(name="psA", bufs=3, space="PSUM") as psp,
            ):
                # gate weight-norm + transpose -> gwT [128d, ND, E]
                gv_sb = ppool.tile([E, D], F32, tag="gv")
                nc.sync.dma_start(gv_sb[:], gatev)
                gg_sb = ppool.tile([E, 1], F32, tag="gg")
                nc.sync.dma_start(gg_sb[:], gateg)
                gparts = ppool.tile([E, 2], F32, tag="gparts")
                for cc in range(2):
                    gscr = ppool.tile([E, 512], F32, tag="wsq", bufs=1)
                    nc.scalar.activation(gscr[:], gv_sb[:, cc * 512:(cc + 1) * 512],
                                         ACT.Square, accum_out=gparts[:, cc:cc + 1])
                gss = ppool.tile([E, 1], F32, tag="gss")
                nc.vector.tensor_reduce(gss[:], gparts[:], axis=AX.X, op=OP.add)
                nc.scalar.sqrt(gss[:], gss[:])
                nc.vector.tensor_scalar_max(gss[:], gss[:], 1e-12)
                grc = ppool.tile([E, 1], F32, tag="grc")
                nc.vector.reciprocal(grc[:], gss[:])
                nc.vector.tensor_tensor(grc[:], grc[:], gg_sb[:], op=OP.mult)
                gwn = ppool.tile([E, D], F32, tag="gwn")
                nc.vector.tensor_scalar_mul(gwn[:], gv_sb[:], grc[:])
                for dc in range(ND):
                    pt = psp.tile([128, 128], F32, tag="t128")
                    nc.tensor.transpose(pt[:, :E], gwn[:, dc * 128:(dc + 1) * 128],
                                        id_sb[:E, :E])
                    nc.vector.tensor_copy(gwT[:, dc, :], pt[:, :E])

                # gating over my token slice, all chunks batched:
                # logits accumulate into ONE psum tile, the top-2 mask math
                # runs as single 3D ops (per-chunk DVE chains pace at ~1us
                # per tiny op and dominated the pre-A2A critical path).
                NTB = TSL // 128
                pgall = psp.tile([128, NTB, E], F32, tag="pgall", bufs=1)
                for tb in range(NTB):
                    xs = ppool.tile([128, D], F32, tag="gx", bufs=2)
                    nc.sync.dma_start(xs[:], xslice[tb * 128:(tb + 1) * 128, :])
                    xt = ppool.tile([128, ND, 128], F32, tag="gxt", bufs=2)
                    for half in range(ND // 4):
                        tp = psp.tile([128, 512], F32, tag="tp4", bufs=2)
                        for k in range(4):
                            dc = half * 4 + k
                            nc.tensor.transpose(
                                tp[:, k * 128:(k + 1) * 128],
                                xs[:, dc * 128:(dc + 1) * 128], id_sb[:])
                        nc.vector.tensor_copy(
                            xt[:, half * 4:half * 4 + 4, :].rearrange(
                                "p a b -> p (a b)"), tp[:])
                    for dc in range(ND):
                        nc.tensor.matmul(
                            pgall[:, tb, :], lhsT=xt[:, dc, :], rhs=gwT[:, dc, :],
                            start=(dc == 0), stop=False,
                        )
                    nc.tensor.matmul(
                        pgall[:, tb, :], lhsT=ones1_sb[:], rhs=gateb_sb[:],
                        start=False, stop=True,
                    )
                lgall = ppool.tile([128, NTB, E], F32, tag="lgall", bufs=1)
                nc.vector.tensor_copy(lgall[:], pgall[:])
                mx1 = ppool.tile([128, NTB, 1], F32, tag="mx1", bufs=1)
                nc.vector.tensor_reduce(mx1[:], lgall[:], axis=AX.X, op=OP.max)
                eq = ppool.tile([128, NTB, E], F32, tag="eq", bufs=1)
                nc.vector.tensor_tensor(eq[:], lgall[:],
                                        mx1[:].to_broadcast([128, NTB, E]),
                                        op=OP.is_equal)
                nc.vector.tensor_scalar_mul(eq[:], eq[:], 1e30)
                nc.vector.tensor_tensor(eq[:], lgall[:], eq[:], op=OP.subtract)
                mx2 = ppool.tile([128, NTB, 1], F32, tag="mx2", bufs=1)
                nc.vector.tensor_reduce(mx2[:], eq[:], axis=AX.X, op=OP.max)
                mk = ppool.tile([128, NTB, E], F32, tag="mk", bufs=1)
                nc.vector.tensor_tensor(mk[:], lgall[:],
                                        mx2[:].to_broadcast([128, NTB, E]),
                                        op=OP.is_ge)
                # reorder free dims to (e, tb), transpose, then a
                # contiguous a2a_in write (a strided DRAM write pattern
                # here was mis-delivered for chunks e>0)
                mkr = ppool.tile([128, E, NTB], F32, tag="mkr", bufs=1)
                nc.vector.tensor_copy(mkr[:], mk[:].rearrange("p a b -> p b a"))
                pmt = psp.tile([128, 128], F32, tag="t128")
                nc.tensor.transpose(pmt[:E * NTB, :],
                                    mkr[:].rearrange("p a b -> p (a b)"),
                                    id_sb[:])
                mkT = ppool.tile([E * NTB, 128], F32, tag="mkT", bufs=1)
                nc.vector.tensor_copy(mkT[:], pmt[:E * NTB, :])
                nc.sync.dma_start(
                    a2a_in.rearrange("(r p) -> r p", p=128), mkT[:])
                nc.gpsimd.collective_compute(
                    "AllToAll", OP.bypass, replica_groups=rg,
                    ins=[a2a_in], outs=[a2a_out],
                )

            # ============ weight pipeline (scl arena, overlaps all) ============
            # w1 row norms first: the first Silu needs s1w.
            for hc in range(NH):
                wt = spool.tile([128, D], BF16, tag="wt", bufs=4)
                nc.scalar.dma_start(wt[:], w1b[hc * 128:(hc + 1) * 128, :])
                scr = spool.tile([128, D], F32, tag="wsq", bufs=1)
                nc.scalar.activation(scr[:], wt[:], ACT.Square,
                                     accum_out=s1w[:, hc:hc + 1])
            nc.scalar.sqrt(s1w[:], s1w[:])
            nc.vector.tensor_scalar_max(s1w[:], s1w[:], 1e-12)
            rc1 = spool.tile([128, NH], F32, tag="rc1", bufs=1)
            nc.vector.reciprocal(rc1[:], s1w[:])
            nc.vector.tensor_tensor(s1w[:], rc1[:], g1w_sb[:], op=OP.mult)

            # w1T transpose-loads (no pool tiles -> no scope coupling)
            for dc in range(ND):
                nc.scalar.dma_start_transpose(
                    w1T[:, dc, :], w1b[:, dc * 128:(dc + 1) * 128])


            # ---- w2T transpose-loads, then w2 norms on DVE from w2T
            # (keeps the ACT engine free for the FFN Silus; no wt2 loads) ----
            for hc in range(NH):
                nc.scalar.dma_start_transpose(
                    w2T[:, hc, :], w2b[:, hc * 128:(hc + 1) * 128])
            s2r = spool.tile([1, D], F32, tag="s2r", bufs=1)
            for j in range(D // 512):
                pr = psps.tile([1, 512], F32, tag=f"spr{j}", bufs=1)
                for hc in range(NH):
                    sq2 = spool.tile([128, 512], BF16, tag="sq2", bufs=3)
                    nc.vector.tensor_tensor(
                        sq2[:], w2T[:, hc, j * 512:(j + 1) * 512],
                        w2T[:, hc, j * 512:(j + 1) * 512], op=OP.mult)
                    nc.tensor.matmul(pr[:], lhsT=onescb_sb[:], rhs=sq2[:],
                                     start=(hc == 0), stop=(hc == NH - 1))
                nc.vector.tensor_copy(s2r[:, j * 512:(j + 1) * 512], pr[:])
            nc.scalar.sqrt(s2r[:], s2r[:])
            nc.vector.tensor_scalar_max(s2r[:], s2r[:], 1e-12)
            nc.vector.reciprocal(s2r[:], s2r[:])
            nc.vector.tensor_tensor(s2r[:], s2r[:], g2r_sb[:], op=OP.mult)
            # replicate the s2 scale row across partitions -> s2t
            for j in range(D // 512):
                pr2 = psps.tile([128, 512], F32, tag="spr0", bufs=1)
                nc.tensor.matmul(pr2[:], lhsT=ones1_sb[:],
                                 rhs=s2r[:, j * 512:(j + 1) * 512],
                                 start=True, stop=True)
                nc.vector.tensor_copy(s2t[:, j * 512:(j + 1) * 512], pr2[:])
            # b2' = b2 / s2 (bias lands before the s2 output scaling);
            # s2r is dead after this, so invert it in place
            nc.vector.reciprocal(s2r[:], s2r[:])
            nc.vector.tensor_tensor(s2r[:], s2r[:], b2r_sb[:], op=OP.mult)
            nc.vector.tensor_copy(b2pb[:], s2r[:])

            # zero the rs_in accumulator (dump rows excluded from RS)
            zt = spool.tile([128, D], BF16, tag="zero", bufs=1)
            nc.gpsimd.memset(zt[:], 0.0)
            for j in range(T // 128):
                nc.sync.dma_start(rs_in[j * 128:(j + 1) * 128, :], zt[:])

            # ============ positions + gather list ============
            with (
                tc.tile_pool(name="mid", bufs=2) as qpool,
                tc.tile_pool(name="psB", bufs=3, space="PSUM") as psp,
            ):
                mrow = qpool.tile([NCH, 128], F32, tag="mrow", bufs=1)
                nc.gpsimd.dma_start(mrow[:],
                                    a2a_out.rearrange("(g p) -> g p", p=128))
                pmk = psp.tile([128, 128], F32, tag="t128")
                nc.tensor.transpose(pmk[:, :NCH], mrow[:], id_sb[:NCH, :NCH])
                mask_sb = qpool.tile([128, NCH], F32, tag="maskf", bufs=1)
                nc.vector.tensor_copy(mask_sb[:], pmk[:, :NCH])
                ppos = psp.tile([128, NCH], F32, tag="t128")
                nc.tensor.matmul(ppos[:], lhsT=u128_sb[:], rhs=mask_sb[:],
                                 start=True, stop=True)
                pref = qpool.tile([128, NCH], F32, tag="pref", bufs=1)
                nc.vector.tensor_copy(pref[:], ppos[:])
                prefT = psp.tile([128, 128], F32, tag="t128")
                nc.tensor.transpose(prefT[:NCH, :], pref[:], id_sb[:])
                tot = qpool.tile([NCH, 1], F32, tag="tot", bufs=1)
                nc.vector.tensor_copy(tot[:], prefT[:NCH, 127:128])
                poff = psp.tile([128, 128], F32, tag="t128")
                nc.tensor.matmul(poff[:NCH, :1], lhsT=uN_sb[:], rhs=tot[:],
                                 start=True, stop=True)
                offs = qpool.tile([NCH, 1], F32, tag="offs", bufs=1)
                nc.vector.tensor_copy(offs[:], poff[:NCH, :1])
                porow = psp.tile([128, 128], F32, tag="t128")
                nc.tensor.transpose(porow[:1, :NCH], offs[:], id_sb[:NCH, :NCH])
                offsrow = qpool.tile([1, NCH], F32, tag="offsrow", bufs=1)
                nc.vector.tensor_copy(offsrow[:], porow[:1, :NCH])
                pbc = psp.tile([128, NCH], F32, tag="t128")
                nc.tensor.matmul(pbc[:], lhsT=ones1_sb[:], rhs=offsrow[:],
                                 start=True, stop=True)
                pos = qpool.tile([128, NCH], F32, tag="pos", bufs=1)
                nc.vector.tensor_tensor(pos[:], pref[:], pbc[:], op=OP.add)
                nc.vector.tensor_tensor(pos[:], pos[:], mask_sb[:],
                                        op=OP.subtract)
                # mask-select: unselected tokens get pos = -BIG (never match)
                nc.vector.tensor_scalar_add(pos[:], pos[:], 1.0e6)
                nc.vector.tensor_tensor(pos[:], pos[:], mask_sb[:], op=OP.mult)
                nc.vector.tensor_scalar_add(pos[:], pos[:], -1.0e6)

                # gather list: gl[j] = (token id + 1) of j-th selected token,
                # via windowed one-hot matmuls; 0 where position j unfilled
                pgl = psp.tile([128, NBJ], F32, tag="pgl", bufs=1)
                for J in range(NBJ):
                    g_lo, g_hi = windows[J]
                    jgJ = qpool.tile([128, 128], F32, tag="jgJ", bufs=2)
                    nc.vector.tensor_scalar_add(jgJ[:], jg_sb[:],
                                                float(128 * J))
                    for g in range(g_lo, g_hi + 1):
                        oh = qpool.tile([128, 128], F32, tag="oh", bufs=8)
                        nc.vector.tensor_tensor(
                            oh[:], pos[:, g:g + 1].to_broadcast([128, 128]),
                            jgJ[:], op=OP.is_equal)
                        nc.tensor.matmul(pgl[:, J:J + 1], lhsT=oh[:],
                                         rhs=tv_sb[:, g:g + 1],
                                         start=(g == g_lo), stop=(g == g_hi))
                # both lists from the raw one-hot result (filled: t+1,
                # unfilled: 0): gather = dump-trick then -1 (reads zero pad
                # rows of xbfp); scatter = t - R0_g per group, so unfilled
                # go negative (dma_scatter_add ignores trailing negatives)
                # and each group's scatter AP can be range-sliced for the
                # chunked ReduceScatter to overlap the FFN.
                gl2 = qpool.tile([128, 2 * NBJ], F32, tag="gl", bufs=1)
                gl = gl2[:, 0:NBJ]
                sl = gl2[:, NBJ:2 * NBJ]
                nc.vector.tensor_copy(gl[:], pgl[:])
                eqz = qpool.tile([128, NBJ], F32, tag="eqz", bufs=1)
                nc.vector.tensor_scalar(eqz[:], gl[:], 0.0, None,
                                        op0=OP.is_equal)
                nc.vector.tensor_tensor(eqz[:], eqz[:],
                                        dumpo_sb[:].to_broadcast([128, NBJ]),
                                        op=OP.mult)
                nc.vector.tensor_tensor(gl[:], gl[:], eqz[:], op=OP.add)
                nc.vector.tensor_scalar_add(gl[:], gl[:], -1.0)
                # scatter list: same entries shifted down by each group's
                # row-range start (its out AP is rs_in[r0 : T+DUMP], so
                # dump targets stay in range and indices stay non-negative
                # -- negative scatter indices wedge the device on re-run)
                for gi, (goff, S) in enumerate(cfg.groups):
                    j0, j1 = goff // 128, (goff + S) // 128
                    nc.vector.tensor_scalar_add(sl[:, j0:j1], gl[:, j0:j1],
                                                float(-grange[gi][0]))

                # on-chip wrap shuffle: [p, J] -> [q, (J, ph)] with p = ph*16+q
                pT = psp.tile([128, 128], F32, tag="t128")
                nc.tensor.transpose(pT[:2 * NBJ, :], gl2[:], id_sb[:])
                glTs = qpool.tile([2 * NBJ, 128], F32, tag="glTs", bufs=1)
                nc.vector.tensor_copy(glTs[:], pT[:2 * NBJ, :])
                glwf = qpool.tile([16, 2 * NBJ, 8], F32, tag="glwf", bufs=1)
                for ph in range(8):
                    pq = psp.tile([128, 128], F32, tag="t128")
                    nc.tensor.transpose(pq[:16, :2 * NBJ],
                                        glTs[:, ph * 16:(ph + 1) * 16],
                                        id_sb[:2 * NBJ, :2 * NBJ])
                    nc.vector.tensor_copy(glwf[:, :, ph], pq[:16, :2 * NBJ])
                glw16 = qpool.tile([16, 2 * (C // 16)], I16, tag="glw16", bufs=1)
                nc.vector.tensor_copy(glw16[:],
                                      glwf[:].rearrange("q g h -> q (g h)"))
                nc.gpsimd.dma_start(glw[0:16, :], glw16[:])
                nc.gpsimd.dma_start(glw[16:32, :], glw[0:16, :])
                nc.gpsimd.dma_start(glw[32:64, :], glw[0:32, :])
                nc.gpsimd.dma_start(glw[64:128, :], glw[0:64, :])

            # ============ expert FFN over compacted tokens ============
            with (
                tc.tile_pool(name="gemm", bufs=2) as gpool,
                tc.tile_pool(name="psC", bufs=2, space="PSUM") as psp,
            ):
                for gi, (goff, S) in enumerate(cfg.groups):
                    xgt = gpool.tile([128, ND, S], BF16, tag="xgt")
                    nc.gpsimd.dma_gather(
                        xgt[:], xbfp, glw[:, goff // 16:(goff + S) // 16],
                        S, S, D, transpose=True,
                    )
                    hT = gpool.tile([128, NH, S], BF16, tag="hT", bufs=1)
                    for hc in range(NH):
                        ph = psp.tile([128, S], F32, tag="ph", bufs=4)
                        for dc in range(ND):
                            nc.tensor.matmul(
                                ph[:], lhsT=w1T[:, dc, hc * 128:(hc + 1) * 128],
                                rhs=xgt[:, dc, :],
                                start=(dc == 0), stop=(dc == ND - 1),
                            )
                        nc.scalar.activation(hT[:, hc, :], ph[:], ACT.Silu,
                                             bias=b1w_sb[:, hc:hc + 1],
                                             scale=s1w[:, hc:hc + 1])
                    og = gpool.tile([128, S // 128, D], BF16, tag="og", bufs=1)
                    for tb in range(S // 128):
                        for dn in range(NDN):
                            po = psp.tile([128, DN], F32, tag="po")
                            for hc in range(NH):
                                nc.tensor.matmul(
                                    po[:], lhsT=hT[:, hc, tb * 128:(tb + 1) * 128],
                                    rhs=w2T[:, hc, dn * DN:(dn + 1) * DN],
                                    start=(hc == 0), stop=False,
                                )
                            nc.tensor.matmul(
                                po[:], lhsT=ones1b_sb[:],
                                rhs=b2pb[:, dn * DN:(dn + 1) * DN],
                                start=False, stop=True,
                            )
                            nc.vector.tensor_tensor(
                                og[:, tb, dn * DN:(dn + 1) * DN], po[:],
                                s2t[:, dn * DN:(dn + 1) * DN], op=OP.mult)
                    r0 = grange[gi][0]
                    nc.gpsimd.dma_scatter_add(
                        rs_in[r0:T + DUMP, :], og[:],
                        glw[:, (C + goff) // 16:(C + goff + S) // 16],
                        S, S, D,
                    )

            if debug:
                d_s1w = nc.dram_tensor("d_s1w", [128, NH], F32,
                                       kind="ExternalOutput").ap()
                nc.sync.dma_start(d_s1w, s1w[:])
                d_s2t = nc.dram_tensor("d_s2t", [128, D], F32,
                                       kind="ExternalOutput").ap()
                nc.sync.dma_start(d_s2t, s2t[:])
                d_b2pb = nc.dram_tensor("d_b2pb", [1, D], BF16,
                                        kind="ExternalOutput").ap()
                nc.sync.dma_start(d_b2pb, b2pb[:])
                d_glw = nc.dram_tensor("d_glw", [128, C // 16], mybir.dt.int16,
                                       kind="ExternalOutput").ap()
                nc.sync.dma_start(d_glw, glw[:])
                d_w1t = nc.dram_tensor("d_w1t", [128, H], BF16,
                                       kind="ExternalOutput").ap()
                nc.sync.dma_start(d_w1t, w1T[:, 3, :])
                d_w2t = nc.dram_tensor("d_w2t", [128, D], BF16,
                                       kind="ExternalOutput").ap()
                nc.sync.dma_start(d_w2t, w2T[:, 5, :])
                d_a2a = nc.dram_tensor("d_a2a", [E * TSL], F32,
                                       kind="ExternalOutput").ap()
                nc.gpsimd.dma_start(out=d_a2a, in_=a2a_out)
                d_rsin = nc.dram_tensor("d_rsin", [256, D], BF16,
                                        kind="ExternalOutput").ap()
                nc.gpsimd.dma_start(out=d_rsin, in_=rs_in[0:256, :])
                d_rsout = nc.dram_tensor("d_rsout", [256, D], BF16,
                                         kind="ExternalOutput").ap()
                nc.gpsimd.dma_start(out=d_rsout, in_=rs_out[0:256, :])

            # ---- ReduceScatter + cast out ----
            RSO = RSC // NCORES
            with tc.tile_pool(name="outp", bufs=2) as opool:
                for k in range(NRS):
                    nc.gpsimd.collective_compute(
                        "ReduceScatter", OP.add, replica_groups=rg,
                        ins=[rs_in[k * RSC:(k + 1) * RSC, :]],
                        outs=[rs_out[k * RSO:(k + 1) * RSO, :]],
                    )
                    # chunk k's shard is final now: cast it out while the
                    # next ReduceScatter chunk runs
                    for j in range(k * RSO // 128, (k + 1) * RSO // 128):
                        rb = opool.tile([128, D], BF16, tag="finb")
                        nc.sync.dma_start(rb[:], rs_out[j * 128:(j + 1) * 128, :])
                        rf = opool.tile([128, D], F32, tag="finf")
                        nc.vector.tensor_copy(rf[:], rb[:])
                        nc.sync.dma_start(out_ext[j * 128:(j + 1) * 128, :],
                                          rf[:])

    return nc


def make_in_maps(cfg: Cfg, x, gate_v, gate_g, gate_b, w1_v, w1_g, b1, w2_v, w2_g, b2):
    """Build the per-core input maps from the full (unsharded) inputs."""
    import ml_dtypes

    T, D, H, E, C = cfg.T, cfg.D, cfg.H, cfg.E, cfg.C
    NH, ND, TSL, NCH = H // 128, D // 128, cfg.TSL, cfg.NCH
    f32 = np.float32
    bf = ml_dtypes.bfloat16
    xf = np.ascontiguousarray(x.reshape(T, D), dtype=f32)
    xbfp = np.zeros((T + cfg.DUMP, D), dtype=bf)
    xbfp[:T] = xf
    u128 = np.triu(np.ones((128, 128), f32))           # u[k,m]=1 iff k<=m
    uN = np.triu(np.ones((NCH, NCH), f32), 1)          # strict upper: k<m
    ones1 = np.ones((1, 128), f32)
    ident = np.eye(128, dtype=f32)
    tvals = np.ascontiguousarray(
        (np.arange(T, dtype=np.int64).reshape(NCH, 128).T + 1).astype(f32))
    jgrid = np.tile(np.arange(128, dtype=f32), (128, 1))
    dumpo = (T + 1 + (np.arange(128) % cfg.DUMP)).astype(f32).reshape(128, 1)
    NCC = 721 + 2 * D

    def pack_cblk(g1wi, b1wi, g2i, gate_bi, b2i):
        cb = np.zeros((128, NCC), f32)
        cb[:, 0:128] = u128
        cb[:, 128:256] = ident
        cb[:, 256:384] = jgrid
        cb[:, 384:384 + NCH] = tvals
        cb[:, 448:449] = dumpo
        cb[:, 449:449 + NH] = g1wi
        cb[:, 481:481 + NH] = b1wi
        cb[0:NCH, 521:521 + NCH] = uN
        cb[0, 585:713] = 1.0
        cb[0, 713:713 + E] = np.asarray(gate_bi, f32).reshape(E)
        cb[0, 721:721 + D] = np.asarray(b2i, f32).reshape(D)
        cb[0, 721 + D:721 + 2 * D] = np.asarray(g2i, f32).reshape(D)
        return cb

    def bf16(a):
        return np.ascontiguousarray(np.asarray(a, f32)).astype(bf)

    def wrap_pc(v, nch):  # [nch*128] -> [128, nch] with v[c*128+p] at [p, c]
        return np.ascontiguousarray(np.asarray(v, f32).reshape(nch, 128).T)

    in_maps = []
    for i in range(cfg.NCORES):
        in_maps.append({
            "xbfp": xbfp,
            "xslice": np.ascontiguousarray(xf[i * TSL:(i + 1) * TSL]),
            "gatev": np.ascontiguousarray(gate_v, dtype=f32),
            "gateg": np.ascontiguousarray(np.asarray(gate_g, f32).reshape(E, 1)),
            "gateb": np.ascontiguousarray(np.asarray(gate_b, f32).reshape(1, E)),
            "w1b": bf16(w1_v[i]),
            "w2b": bf16(w2_v[i]),
            "cblk": pack_cblk(wrap_pc(w1_g[i], NH), wrap_pc(b1[i], NH),
                              w2_g[i], gate_b, b2[i]),
            "ones1b": bf16(ones1),
            "onescb": bf16(np.ones((128, 1), f32)),
        })
    return in_maps


_COMPILED = {}


def compute_routing(cfg: Cfg, x, gate_v, gate_g, gate_b):
    """Host-side routing (f64) -> per-chunk counts for windows/capacity."""
    T, D, E = cfg.T, cfg.D, cfg.E
    NCH = cfg.NCH
    xf = np.asarray(x, np.float64).reshape(T, D)
    gv = np.asarray(gate_v, np.float64)
    gw = gv / np.maximum(np.sqrt((gv * gv).sum(-1, keepdims=True)), 1e-12)
    gw = gw * np.asarray(gate_g, np.float64).reshape(E, 1)
    logits = xf @ gw.T + np.asarray(gate_b, np.float64).reshape(1, E)
    part = np.argpartition(-logits, 2, axis=1)[:, :2]
    cnts = np.zeros((E, NCH), np.int64)
    for e in range(E):
        m = (part == e).any(1)
        cnts[e] = m.reshape(NCH, 128).sum(1)
    return cnts


def pick_capacity(cfg: Cfg, cnts, margin=33):
    maxcnt = int(cnts.sum(1).max())
    C = ((maxcnt + margin + 127) // 128) * 128
    return C


def compute_windows(cfg: Cfg, cnts, margin=32):
    """Conservative (J -> chunk range) windows for the gather-list build."""
    C, NCH, NBJ = cfg.C, cfg.NCH, cfg.C // 128
    lo = [NCH] * NBJ
    hi = [-1] * NBJ
    for e in range(cfg.E):
        cnt = cnts[e]
        offs = np.concatenate([[0], np.cumsum(cnt)[:-1]])
        for g in range(NCH):
            a = max(0, int(offs[g]) - margin)
            b = min(C, int(offs[g] + cnt[g]) + margin)
            if b <= a:
                b = a + 1
            for J in range(a // 128, min(NBJ - 1, (b - 1) // 128) + 1):
                lo[J] = min(lo[J], g)
                hi[J] = max(hi[J], g)
    return tuple((min(lo[J], NCH - 1), max(hi[J], min(lo[J], NCH - 1)))
                 for J in range(NBJ))


def get_compiled(cfg: Cfg, windows=None):
    key = (cfg.T, cfg.D, cfg.H, cfg.E, cfg.C, tuple(cfg.groups), windows)
    if key not in _COMPILED:
        nc = bacc.Bacc("TRN2", target_bir_lowering=False, debug=False,
                       num_devices=cfg.NCORES)
        build_moe(nc, cfg, windows)
        nc.compile()
        _COMPILED[key] = nc
    return _COMPILED[key]


def plan(x, gate_v, gate_g, gate_b):
    cfg0 = Cfg()
    cnts = compute_routing(cfg0, x, gate_v, gate_g, gate_b)
    C = pick_capacity(cfg0, cnts)
    cfg = Cfg(C=C)
    windows = compute_windows(cfg, cnts)
    return cfg, windows


def kernel(x, gate_v, gate_g, gate_b, w1_v, w1_g, b1, w2_v, w2_g, b2):
    from concourse.bass_utils import run_bass_kernel_spmd

    cfg, windows = plan(x, gate_v, gate_g, gate_b)
    nc = get_compiled(cfg, windows)
    in_maps = make_in_maps(cfg, np.asarray(x), np.asarray(gate_v),
                           np.asarray(gate_g), np.asarray(gate_b),
                           np.asarray(w1_v), np.asarray(w1_g), np.asarray(b1),
                           np.asarray(w2_v), np.asarray(w2_g), np.asarray(b2))
    res = run_bass_kernel_spmd(nc, in_maps, core_ids=list(range(cfg.NCORES)))
    shards = [res.results[i]["out"] for i in range(cfg.NCORES)]
    out = unpermute(cfg, np.stack(shards, axis=0))
    B, S_, D_ = x.shape
    return out.reshape(B, S_, D_)


def unpermute(cfg: Cfg, shards):
    """Chunked-RS row order -> token order.

    RS chunk k hands rank i rows [2048k + 256i, 2048k + 256(i+1)) of the
    token axis, stored at rs_out rows [256k, 256(k+1)).
    """
    arr = np.asarray(shards, np.float32)       # [8, TSL, D]
    n, tsl, d = arr.shape
    nrs = cfg.NRS
    blk = tsl // nrs
    return arr.reshape(n, nrs, blk, d).transpose(1, 0, 2, 3).reshape(n * tsl, d)


# revision 56
# speedup vs baseline: 1.1672x; 1.1672x over previous
"""Expert-parallel MoE kernel for one TRN2 chip (8 NeuronCores).

Strategy (expert-parallel, top-2 sparse):
  - core e owns expert e's weights. Weights arrive as bf16 copies of
    w{1,2}_v; they are loaded into SBUF pre-transposed via HWDGE
    DMA-transpose. Weight-norm is never applied to the big tensors:
    GEMM1 folds the per-row scale s1=g1/||v1|| into the Silu
    activation's per-partition `scale`, GEMM2 folds s2=g2/||v2|| into
    w2T once (per-column DVE multiply). The scales themselves are
    computed on device from the bf16 weights (square + ones-matmul
    partition reduction).
  - gating is data-parallel and runs FIRST: core i computes top-2
    masks for its T/8 token slice with exact fp32 matmuls (so routing
    never flips), then an AllToAll exchanges per-expert mask rows so
    every core holds the full-T mask for ITS expert. Weight DMA
    overlaps with gating/positions on the DMA engines.
  - token compaction: matmul prefix-sums turn the mask into global
    compacted positions; the gather list is built with windowed
    one-hot matmuls (windows from host-side routing, margin 64).
  - x arrives as a zero-padded bf16 copy; expert FFN gathers its
    compacted tokens transposed via dma_gather, runs dense GEMMs
    (bf16 compute / fp32 accumulate), and scatter-adds outputs back
    to token order (dropped tokens land in dump rows).
  - one ReduceScatter sums the 8 expert partials; each core emits its
    T/8-row shard via a casting DRAM->DRAM DMA.
"""

import numpy as np

import concourse.bass as bass
import concourse.mybir as mybir
import concourse.tile as tile
from concourse import bacc
from concourse.library_config import mlp

F32 = mybir.dt.float32
BF16 = mybir.dt.bfloat16
I16 = mybir.dt.int16

AX = mybir.AxisListType
OP = mybir.AluOpType
ACT = mybir.ActivationFunctionType


class Cfg:
    def __init__(self, T=8192, D=1024, H=4096, E=8, NCORES=8, C=2304,
                 use_silu=True):
        self.T, self.D, self.H, self.E = T, D, H, E
        self.NCORES = NCORES
        self.C = C               # per-expert token capacity (multiple of 128)
        self.TSL = T // NCORES   # tokens per core slice
        self.NCH = T // 128      # 128-token chunks
        self.DUMP = 64           # spread dump rows
        self.NRS = 4             # ReduceScatter chunks
        self.use_silu = use_silu
        # FFN token groups: 256-wide, with a 128 tail when C allows
        self.groups = []
        off = 0
        while off < C:
            s = 256 if C - off >= 256 else 128
            self.groups.append((off, s))
            off += s
        assert C % 128 == 0
        assert T % (NCORES * 128) == 0 and D % 128 == 0 and H % 128 == 0
        assert self.NCH <= 128


def build_moe(nc, cfg: Cfg, windows=None, debug=False):
    T, D, H, E, C = cfg.T, cfg.D, cfg.H, cfg.E, cfg.C
    TSL, NCH, DUMP = cfg.TSL, cfg.NCH, cfg.DUMP
    ND = D // 128   # D chunks of 128
    NH = H // 128   # H chunks of 128
    NDN = max(1, D // 512)       # GEMM2 output-column tiles
    DN = min(512, D)
    NCORES = cfg.NCORES
    NBJ = C // 128               # position blocks for gather-list build
    if windows is None:
        windows = [(J, NCH - 1) for J in range(NBJ)]
    # per-FFN-group token-row range touched by its scatter (from windows)
    grange = []
    for goff, S in cfg.groups:
        js = range(goff // 128, (goff + S) // 128)
        r0 = min(windows[J][0] for J in js) * 128
        r1 = (max(windows[J][1] for J in js) + 1) * 128
        grange.append((r0, r1))
    NRS = cfg.NRS                 # ReduceScatter chunks
    RSC = T // NRS

    # ---------------- kernel I/O ----------------
    xbfp = nc.dram_tensor("xbfp", [T + DUMP, D], BF16, kind="ExternalInput").ap()
    xslice = nc.dram_tensor("xslice", [TSL, D], F32, kind="ExternalInput").ap()
    gatev = nc.dram_tensor("gatev", [E, D], F32, kind="ExternalInput").ap()
    gateg = nc.dram_tensor("gateg", [E, 1], F32, kind="ExternalInput").ap()
    w1b = nc.dram_tensor("w1b", [H, D], BF16, kind="ExternalInput").ap()
    w2b = nc.dram_tensor("w2b", [D, H], BF16, kind="ExternalInput").ap()
    NCC = 721 + 2 * D
    cblk = nc.dram_tensor("cblk", [128, NCC], F32, kind="ExternalInput").ap()
    ones1b = nc.dram_tensor("ones1b", [1, 128], BF16, kind="ExternalInput").ap()
    onescb = nc.dram_tensor("onescb", [128, 1], BF16, kind="ExternalInput").ap()
    out_ext = nc.dram_tensor("out", [TSL, D], F32, kind="ExternalOutput").ap()

    # ---------------- internal DRAM ----------------
    a2a_in = nc.dram_tensor("a2a_in", [E * TSL], F32).ap()
    a2a_out = nc.dram_tensor("a2a_out", [E * TSL], F32).ap()
    rs_in = nc.dram_tensor("rs_in", [T + DUMP, D], BF16).ap()
    rs_out = nc.dram_tensor("rs_out", [TSL, D], BF16).ap()

    rg = [list(range(NCORES))]

    with tile.TileContext(nc) as tc:
        nc.gpsimd.load_library(mlp)

        # Pool layout: cpool/wpool/spool stay open for the whole kernel
        # (two parallel arenas); prep -> mid -> gemm are sequential scopes
        # reusing one arena in their natural time order. Scope boundaries
        # serialize (a later pool's first tile waits for every release of
        # the earlier one), so only naturally-ordered phases share space.
        with (
            tc.tile_pool(name="consts", bufs=1) as cpool,
            tc.tile_pool(name="wbig", bufs=1) as wpool,
            tc.tile_pool(name="scl", bufs=2) as spool,
            tc.tile_pool(name="psS", bufs=1, space="PSUM") as psps,
        ):
            # ---- constants: one packed DMA (tiny DMAs serialize at ~2.5us
            # completion latency each and stall the gating chain) ----
            cb = cpool.tile([128, NCC], F32)
            nc.sync.dma_start(cb[:], cblk)
            u128_sb = cb[:, 0:128]
            id_sb = cb[:, 128:256]
            jg_sb = cb[:, 256:384]
            tv_sb = cb[:, 384:384 + NCH]
            dumpo_sb = cb[:, 448:449]
            g1w_sb = cb[:, 449:449 + NH]
            b1w_sb = cb[:, 481:481 + NH]
            g2r_sb = cb[0:1, 721 + D:721 + 2 * D]
            uN_sb = cb[0:NCH, 521:521 + NCH]
            ones1_sb = cb[0:1, 585:713]
            gateb_sb = cb[0:1, 713:713 + E]
            b2r_sb = cb[0:1, 721:721 + D]
            ones1b_sb = cpool.tile([1, 128], BF16)
            nc.sync.dma_start(ones1b_sb[:], ones1b)
            onescb_sb = cpool.tile([128, 1], BF16)
            nc.sync.dma_start(onescb_sb[:], onescb)
            gwT = cpool.tile([128, ND, E], F32)       # normalized gate W^T
            s1w = cpool.tile([128, NH], F32)          # GEMM1 activation scale
            s2t = cpool.tile([128, D], F32)           # GEMM2 per-column scale
            b2pb = cpool.tile([1, D], BF16)           # b2 / s2, bf16
            glw = cpool.tile([128, 2 * (C // 16)], I16)  # gather+scatter lists
            w1T = wpool.tile([128, ND, H], BF16)      # [d, dc, h] = w1_v^T (bf16)
            w2T = wpool.tile([128, NH, D], BF16)      # [h, hc, d] = w2_v^T (bf16)

            # ================ gating (exact f32) -> maskT, A2A ================
            with (
                tc.tile_pool(name="prep", bufs=2) as ppool,
                tc.tile_pool(name="psA", bufs=3, space="PSUM") as psp,
            ):
                # gate weight-norm + transpose -> gwT [128d, ND, E]
                gv_sb = ppool.tile([E, D], F32, tag="gv")
                nc.sync.dma_start(gv_sb[:], gatev)
                gg_sb = ppool.tile([E, 1], F32, tag="gg")
                nc.sync.dma_start(gg_sb[:], gateg)
                gparts = ppool.tile([E, 2], F32, tag="gparts")
                for cc in range(2):
                    gscr = ppool.tile([E, 512], F32, tag="wsq", bufs=1)
                    nc.scalar.activation(gscr[:], gv_sb[:, cc * 512:(cc + 1) * 512],
                                         ACT.Square, accum_out=gparts[:, cc:cc + 1])
                gss = ppool.tile([E, 1], F32, tag="gss")
                nc.vector.tensor_reduce(gss[:], gparts[:], axis=AX.X, op=OP.add)
                nc.scalar.sqrt(gss[:], gss[:])
                nc.vector.tensor_scalar_max(gss[:], gss[:], 1e-12)
                grc = ppool.tile([E, 1], F32, tag="grc")
                nc.vector.reciprocal(grc[:], gss[:])
                nc.vector.tensor_tensor(grc[:], grc[:], gg_sb[:], op=OP.mult)
                gwn = ppool.tile([E, D], F32, tag="gwn")
                nc.vector.tensor_scalar_mul(gwn[:], gv_sb[:], grc[:])
                for dc in range(ND):
                    pt = psp.tile([128, 128], F32, tag="t128")
                    nc.tensor.transpose(pt[:, :E], gwn[:, dc * 128:(dc + 1) * 128],
                                        id_sb[:E, :E])
                    nc.vector.tensor_copy(gwT[:, dc, :], pt[:, :E])

                # gating over my token slice, all chunks batched:
                # logits accumulate into ONE psum tile, the top-2 mask math
                # runs as single 3D ops (per-chunk DVE chains pace at ~1us
                # per tiny op and dominated the pre-A2A critical path).
                NTB = TSL // 128
                pgall = psp.tile([128, NTB, E], F32, tag="pgall", bufs=1)
                for tb in range(NTB):
                    xs = ppool.tile([128, D], F32, tag="gx", bufs=2)
                    nc.sync.dma_start(xs[:], xslice[tb * 128:(tb + 1) * 128, :])
                    xt = ppool.tile([128, ND, 128], F32, tag="gxt", bufs=2)
                    for half in range(ND // 4):
                        tp = psp.tile([128, 512], F32, tag="tp4", bufs=2)
                        for k in range(4):
                            dc = half * 4 + k
                            nc.tensor.transpose(
                                tp[:, k * 128:(k + 1) * 128],
                                xs[:, dc * 128:(dc + 1) * 128], id_sb[:])
                        nc.vector.tensor_copy(
                            xt[:, half * 4:half * 4 + 4, :].rearrange(
                                "p a b -> p (a b)"), tp[:])
                    for dc in range(ND):
                        nc.tensor.matmul(
                            pgall[:, tb, :], lhsT=xt[:, dc, :], rhs=gwT[:, dc, :],
                            start=(dc == 0), stop=False,
                        )
                    nc.tensor.matmul(
                        pgall[:, tb, :], lhsT=ones1_sb[:], rhs=gateb_sb[:],
                        start=False, stop=True,
                    )
                lgall = ppool.tile([128, NTB, E], F32, tag="lgall", bufs=1)
                nc.vector.tensor_copy(lgall[:], pgall[:])
                mx1 = ppool.tile([128, NTB, 1], F32, tag="mx1", bufs=1)
                nc.vector.tensor_reduce(mx1[:], lgall[:], axis=AX.X, op=OP.max)
                eq = ppool.tile([128, NTB, E], F32, tag="eq", bufs=1)
                nc.vector.tensor_tensor(eq[:], lgall[:],
                                        mx1[:].to_broadcast([128, NTB, E]),
                                        op=OP.is_equal)
                nc.vector.tensor_scalar_mul(eq[:], eq[:], 1e30)
                nc.vector.tensor_tensor(eq[:], lgall[:], eq[:], op=OP.subtract)
                mx2 = ppool.tile([128, NTB, 1], F32, tag="mx2", bufs=1)
                nc.vector.tensor_reduce(mx2[:], eq[:], axis=AX.X, op=OP.max)
                mk = ppool.tile([128, NTB, E], F32, tag="mk", bufs=1)
                nc.vector.tensor_tensor(mk[:], lgall[:],
                                        mx2[:].to_broadcast([128, NTB, E]),
                                        op=OP.is_ge)
                # reorder free dims to (e, tb), transpose, then a
                # contiguous a2a_in write (a strided DRAM write pattern
                # here was mis-delivered for chunks e>0)
                mkr = ppool.tile([128, E, NTB], F32, tag="mkr", bufs=1)
                nc.vector.tensor_copy(mkr[:], mk[:].rearrange("p a b -> p b a"))
                pmt = psp.tile([128, 128], F32, tag="t128")
                nc.tensor.transpose(pmt[:E * NTB, :],
                                    mkr[:].rearrange("p a b -> p (a b)"),
                                    id_sb[:])
                mkT = ppool.tile([E * NTB, 128], F32, tag="mkT", bufs=1)
                nc.vector.tensor_copy(mkT[:], pmt[:E * NTB, :])
                nc.sync.dma_start(
                    a2a_in.rearrange("(r p) -> r p", p=128), mkT[:])
                nc.gpsimd.collective_compute(
                    "AllToAll", OP.bypass, replica_groups=rg,
                    ins=[a2a_in], outs=[a2a_out],
                )

            # ============ weight pipeline (scl arena, overlaps all) ============
            # w1 row norms first: the first Silu needs s1w.
            for hc in range(NH):
                wt = spool.tile([128, D], BF16, tag="wt", bufs=4)
                nc.scalar.dma_start(wt[:], w1b[hc * 128:(hc + 1) * 128, :])
                scr = spool.tile([128, D], F32, tag="wsq", bufs=1)
                nc.scalar.activation(scr[:], wt[:], ACT.Square,
                                     accum_out=s1w[:, hc:hc + 1])
            nc.scalar.sqrt(s1w[:], s1w[:])
            nc.vector.tensor_scalar_max(s1w[:], s1w[:], 1e-12)
            rc1 = spool.tile([128, NH], F32, tag="rc1", bufs=1)
            nc.vector.reciprocal(rc1[:], s1w[:])
            nc.vector.tensor_tensor(s1w[:], rc1[:], g1w_sb[:], op=OP.mult)

            # w1T transpose-loads (no pool tiles -> no scope coupling)
            for dc in range(ND):
                nc.scalar.dma_start_transpose(
                    w1T[:, dc, :], w1b[:, dc * 128:(dc + 1) * 128])


            # ---- w2T transpose-loads, then w2 norms on DVE from w2T
            # (keeps the ACT engine free for the FFN Silus; no wt2 loads) ----
            for hc in range(NH):
                nc.scalar.dma_start_transpose(
                    w2T[:, hc, :], w2b[:, hc * 128:(hc + 1) * 128])
            s2r = spool.tile([1, D], F32, tag="s2r", bufs=1)
            for j in range(D // 512):
                pr = psps.tile([1, 512], F32, tag=f"spr{j}", bufs=1)
                for hc in range(NH):
                    sq2 = spool.tile([128, 512], BF16, tag="sq2", bufs=3)
                    nc.vector.tensor_tensor(
                        sq2[:], w2T[:, hc, j * 512:(j + 1) * 512],
                        w2T[:, hc, j * 512:(j + 1) * 512], op=OP.mult)
                    nc.tensor.matmul(pr[:], lhsT=onescb_sb[:], rhs=sq2[:],
                                     start=(hc == 0), stop=(hc == NH - 1))
                nc.vector.tensor_copy(s2r[:, j * 512:(j + 1) * 512], pr[:])
            nc.scalar.sqrt(s2r[:], s2r[:])
            nc.vector.tensor_scalar_max(s2r[:], s2r[:], 1e-12)
            nc.vector.reciprocal(s2r[:], s2r[:])
            nc.vector.tensor_tensor(s2r[:], s2r[:], g2r_sb[:], op=OP.mult)
            # replicate the s2 scale row across partitions -> s2t
            for j in range(D // 512):
                pr2 = psps.tile([128, 512], F32, tag="spr0", bufs=1)
                nc.tensor.matmul(pr2[:], lhsT=ones1_sb[:],
                                 rhs=s2r[:, j * 512:(j + 1) * 512],
                                 start=True, stop=True)
                nc.vector.tensor_copy(s2t[:, j * 512:(j + 1) * 512], pr2[:])
            # b2' = b2 / s2 (bias lands before the s2 output scaling);
            # s2r is dead after this, so invert it in place
            nc.vector.reciprocal(s2r[:], s2r[:])
            nc.vector.tensor_tensor(s2r[:], s2r[:], b2r_sb[:], op=OP.mult)
            nc.vector.tensor_copy(b2pb[:], s2r[:])

            # zero the rs_in accumulator (dump rows excluded from RS)
            zt = spool.tile([128, D], BF16, tag="zero", bufs=1)
            nc.gpsimd.memset(zt[:], 0.0)
            for j in range(T // 128):
                nc.sync.dma_start(rs_in[j * 128:(j + 1) * 128, :], zt[:])

            # ============ positions + gather list ============
            with (
                tc.tile_pool(name="mid", bufs=2) as qpool,
                tc.tile_pool(name="psB", bufs=3, space="PSUM") as psp,
            ):
                mrow = qpool.tile([NCH, 128], F32, tag="mrow", bufs=1)
                nc.gpsimd.dma_start(mrow[:],
                                    a2a_out.rearrange("(g p) -> g p", p=128))
                pmk = psp.tile([128, 128], F32, tag="t128")
                nc.tensor.transpose(pmk[:, :NCH], mrow[:], id_sb[:NCH, :NCH])
                mask_sb = qpool.tile([128, NCH], F32, tag="maskf", bufs=1)
                nc.vector.tensor_copy(mask_sb[:], pmk[:, :NCH])
                ppos = psp.tile([128, NCH], F32, tag="t128")
                nc.tensor.matmul(ppos[:], lhsT=u128_sb[:], rhs=mask_sb[:],
                                 start=True, stop=True)
                pref = qpool.tile([128, NCH], F32, tag="pref", bufs=1)
                nc.vector.tensor_copy(pref[:], ppos[:])
                prefT = psp.tile([128, 128], F32, tag="t128")
                nc.tensor.transpose(prefT[:NCH, :], pref[:], id_sb[:])
                tot = qpool.tile([NCH, 1], F32, tag="tot", bufs=1)
                nc.vector.tensor_copy(tot[:], prefT[:NCH, 127:128])
                poff = psp.tile([128, 128], F32, tag="t128")
                nc.tensor.matmul(poff[:NCH, :1], lhsT=uN_sb[:], rhs=tot[:],
                                 start=True, stop=True)
                offs = qpool.tile([NCH, 1], F32, tag="offs", bufs=1)
                nc.vector.tensor_copy(offs[:], poff[:NCH, :1])
                porow = psp.tile([128, 128], F32, tag="t128")
                nc.tensor.transpose(porow[:1, :NCH], offs[:], id_sb[:NCH, :NCH])
                offsrow = qpool.tile([1, NCH], F32, tag="offsrow", bufs=1)
                nc.vector.tensor_copy(offsrow[:], porow[:1, :NCH])
                pbc = psp.tile([128, NCH], F32, tag="t128")
                nc.tensor.matmul(pbc[:], lhsT=ones1_sb[:], rhs=offsrow[:],
                                 start=True, stop=True)
                pos = qpool.tile([128, NCH], F32, tag="pos", bufs=1)
                nc.vector.tensor_tensor(pos[:], pref[:], pbc[:], op=OP.add)
                nc.vector.tensor_tensor(pos[:], pos[:], mask_sb[:],
                                        op=OP.subtract)
                # mask-select: unselected tokens get pos = -BIG (never match)
                nc.vector.tensor_scalar_add(pos[:], pos[:], 1.0e6)
                nc.vector.tensor_tensor(pos[:], pos[:], mask_sb[:], op=OP.mult)
                nc.vector.tensor_scalar_add(pos[:], pos[:], -1.0e6)

                # gather list: gl[j] = (token id + 1) of j-th selected token,
                # via windowed one-hot matmuls; 0 where position j unfilled
                pgl = psp.tile([128, NBJ], F32, tag="pgl", bufs=1)
                for J in range(NBJ):
                    g_lo, g_hi = windows[J]
                    jgJ = qpool.tile([128, 128], F32, tag="jgJ", bufs=2)
                    nc.vector.tensor_scalar_add(jgJ[:], jg_sb[:],
                                                float(128 * J))
                    for g in range(g_lo, g_hi + 1):
                        oh = qpool.tile([128, 128], F32, tag="oh", bufs=8)
                        nc.vector.tensor_tensor(
                            oh[:], pos[:, g:g + 1].to_broadcast([128, 128]),
                            jgJ[:], op=OP.is_equal)
                        nc.tensor.matmul(pgl[:, J:J + 1], lhsT=oh[:],
                                         rhs=tv_sb[:, g:g + 1],
                                         start=(g == g_lo), stop=(g == g_hi))
                # both lists from the raw one-hot result (filled: t+1,
                # unfilled: 0): gather = dump-trick then -1 (reads zero pad
                # rows of xbfp); scatter = t - R0_g per group, so unfilled
                # go negative (dma_scatter_add ignores trailing negatives)
                # and each group's scatter AP can be range-sliced for the
                # chunked ReduceScatter to overlap the FFN.
                gl2 = qpool.tile([128, 2 * NBJ], F32, tag="gl", bufs=1)
                gl = gl2[:, 0:NBJ]
                sl = gl2[:, NBJ:2 * NBJ]
                nc.vector.tensor_copy(gl[:], pgl[:])
                eqz = qpool.tile([128, NBJ], F32, tag="eqz", bufs=1)
                nc.vector.tensor_scalar(eqz[:], gl[:], 0.0, None,
                                        op0=OP.is_equal)
                nc.vector.tensor_tensor(eqz[:], eqz[:],
                                        dumpo_sb[:].to_broadcast([128, NBJ]),
                                        op=OP.mult)
                nc.vector.tensor_tensor(gl[:], gl[:], eqz[:], op=OP.add)
                nc.vector.tensor_scalar_add(gl[:], gl[:], -1.0)
                # scatter list: same entries shifted down by each group's
                # row-range start (its out AP is rs_in[r0 : T+DUMP], so
                # dump targets stay in range and indices stay non-negative
                # -- negative scatter indices wedge the device on re-run)
                for gi, (goff, S) in enumerate(cfg.groups):
                    j0, j1 = goff // 128, (goff + S) // 128
                    nc.vector.tensor_scalar_add(sl[:, j0:j1], gl[:, j0:j1],
                                                float(-grange[gi][0]))

                # on-chip wrap shuffle: [p, J] -> [q, (J, ph)] with p = ph*16+q
                pT = psp.tile([128, 128], F32, tag="t128")
                nc.tensor.transpose(pT[:2 * NBJ, :], gl2[:], id_sb[:])
                glTs = qpool.tile([2 * NBJ, 128], F32, tag="glTs", bufs=1)
                nc.vector.tensor_copy(glTs[:], pT[:2 * NBJ, :])
                glwf = qpool.tile([16, 2 * NBJ, 8], F32, tag="glwf", bufs=1)
                for ph in range(8):
                    pq = psp.tile([128, 128], F32, tag="t128")
                    nc.tensor.transpose(pq[:16, :2 * NBJ],
                                        glTs[:, ph * 16:(ph + 1) * 16],
                                        id_sb[:2 * NBJ, :2 * NBJ])
                    nc.vector.tensor_copy(glwf[:, :, ph], pq[:16, :2 * NBJ])
                glw16 = qpool.tile([16, 2 * (C // 16)], I16, tag="glw16", bufs=1)
                nc.vector.tensor_copy(glw16[:],
                                      glwf[:].rearrange("q g h -> q (g h)"))
                nc.gpsimd.dma_start(glw[0:16, :], glw16[:])
                nc.gpsimd.dma_start(glw[16:32, :], glw[0:16, :])
                nc.gpsimd.dma_start(glw[32:64, :], glw[0:32, :])
                nc.gpsimd.dma_start(glw[64:128, :], glw[0:64, :])

            # ============ expert FFN over compacted tokens ============
            with (
                tc.tile_pool(name="gemm", bufs=2) as gpool,
                tc.tile_pool(name="psC", bufs=2, space="PSUM") as psp,
            ):
                for gi, (goff, S) in enumerate(cfg.groups):
                    xgt = gpool.tile([128, ND, S], BF16, tag="xgt")
                    nc.gpsimd.dma_gather(
                        xgt[:], xbfp, glw[:, goff // 16:(goff + S) // 16],
                        S, S, D, transpose=True,
                    )
                    hT = gpool.tile([128, NH, S], BF16, tag="hT", bufs=1)
                    for hc in range(NH):
                        ph = psp.tile([128, S], F32, tag="ph", bufs=4)
                        for dc in range(ND):
                            nc.tensor.matmul(
                                ph[:], lhsT=w1T[:, dc, hc * 128:(hc + 1) * 128],
                                rhs=xgt[:, dc, :],
                                start=(dc == 0), stop=(dc == ND - 1),
                            )
                        nc.scalar.activation(hT[:, hc, :], ph[:], ACT.Silu,
                                             bias=b1w_sb[:, hc:hc + 1],
                                             scale=s1w[:, hc:hc + 1])
                    og = gpool.tile([128, S // 128, D], BF16, tag="og", bufs=1)
                    for tb in range(S // 128):
                        for dn in range(NDN):
                            po = psp.tile([128, DN], F32, tag="po")
                            for hc in range(NH):
                                nc.tensor.matmul(
                                    po[:], lhsT=hT[:, hc, tb * 128:(tb + 1) * 128],
                                    rhs=w2T[:, hc, dn * DN:(dn + 1) * DN],
                                    start=(hc == 0), stop=False,
                                )
                            nc.tensor.matmul(
                                po[:], lhsT=ones1b_sb[:],
                                rhs=b2pb[:, dn * DN:(dn + 1) * DN],
                                start=False, stop=True,
                            )
                            nc.vector.tensor_tensor(
                                og[:, tb, dn * DN:(dn + 1) * DN], po[:],
                                s2t[:, dn * DN:(dn + 1) * DN], op=OP.mult)
                    r0 = grange[gi][0]
                    nc.gpsimd.dma_scatter_add(
                        rs_in[r0:T + DUMP, :], og[:],
                        glw[:, (C + goff) // 16:(C + goff + S) // 16],
                        S, S, D,
                    )

            if debug:
                d_s1w = nc.dram_tensor("d_s1w", [128, NH], F32,
                                       kind="ExternalOutput").ap()
                nc.sync.dma_start(d_s1w, s1w[:])
                d_s2t = nc.dram_tensor("d_s2t", [128, D], F32,
                                       kind="ExternalOutput").ap()
                nc.sync.dma_start(d_s2t, s2t[:])
                d_b2pb = nc.dram_tensor("d_b2pb", [1, D], BF16,
                                        kind="ExternalOutput").ap()
                nc.sync.dma_start(d_b2pb, b2pb[:])
                d_glw = nc.dram_tensor("d_glw", [128, C // 16], mybir.dt.int16,
                                       kind="ExternalOutput").ap()
                nc.sync.dma_start(d_glw, glw[:])
                d_w1t = nc.dram_tensor("d_w1t", [128, H], BF16,
                                       kind="ExternalOutput").ap()
                nc.sync.dma_start(d_w1t, w1T[:, 3, :])
                d_w2t = nc.dram_tensor("d_w2t", [128, D], BF16,
                                       kind="ExternalOutput").ap()
                nc.sync.dma_start(d_w2t, w2T[:, 5, :])
                d_a2a = nc.dram_tensor("d_a2a", [E * TSL], F32,
                                       kind="ExternalOutput").ap()
                nc.gpsimd.dma_start(out=d_a2a, in_=a2a_out)
                d_rsin = nc.dram_tensor("d_rsin", [256, D], BF16,
                                        kind="ExternalOutput").ap()
                nc.gpsimd.dma_start(out=d_rsin, in_=rs_in[0:256, :])
                d_rsout = nc.dram_tensor("d_rsout", [256, D], BF16,
                                         kind="ExternalOutput").ap()
                nc.gpsimd.dma_start(out=d_rsout, in_=rs_out[0:256, :])

            # ---- ReduceScatter + cast out ----
            RSO = RSC // NCORES
            with tc.tile_pool(name="outp", bufs=2) as opool:
                for k in range(NRS):
                    nc.gpsimd.collective_compute(
                        "ReduceScatter", OP.add, replica_groups=rg,
                        ins=[rs_in[k * RSC:(k + 1) * RSC, :]],
                        outs=[rs_out[k * RSO:(k + 1) * RSO, :]],
                    )
                    # chunk k's shard is final now: cast it out while the
                    # next ReduceScatter chunk runs
                    for j in range(k * RSO // 128, (k + 1) * RSO // 128):
                        rb = opool.tile([128, D], BF16, tag="finb")
                        nc.sync.dma_start(rb[:], rs_out[j * 128:(j + 1) * 128, :])
                        rf = opool.tile([128, D], F32, tag="finf")
                        nc.vector.tensor_copy(rf[:], rb[:])
                        nc.sync.dma_start(out_ext[j * 128:(j + 1) * 128, :],
                                          rf[:])

    return nc


def make_in_maps(cfg: Cfg, x, gate_v, gate_g, gate_b, w1_v, w1_g, b1, w2_v, w2_g, b2):
    """Build the per-core input maps from the full (unsharded) inputs."""
    import ml_dtypes

    T, D, H, E, C = cfg.T, cfg.D, cfg.H, cfg.E, cfg.C
    NH, ND, TSL, NCH = H // 128, D // 128, cfg.TSL, cfg.NCH
    f32 = np.float32
    bf = ml_dtypes.bfloat16
    xf = np.ascontiguousarray(x.reshape(T, D), dtype=f32)
    xbfp = np.zeros((T + cfg.DUMP, D), dtype=bf)
    xbfp[:T] = xf
    u128 = np.triu(np.ones((128, 128), f32))           # u[k,m]=1 iff k<=m
    uN = np.triu(np.ones((NCH, NCH), f32), 1)          # strict upper: k<m
    ones1 = np.ones((1, 128), f32)
    ident = np.eye(128, dtype=f32)
    tvals = np.ascontiguousarray(
        (np.arange(T, dtype=np.int64).reshape(NCH, 128).T + 1).astype(f32))
    jgrid = np.tile(np.arange(128, dtype=f32), (128, 1))
    dumpo = (T + 1 + (np.arange(128) % cfg.DUMP)).astype(f32).reshape(128, 1)
    NCC = 721 + 2 * D

    def pack_cblk(g1wi, b1wi, g2i, gate_bi, b2i):
        cb = np.zeros((128, NCC), f32)
        cb[:, 0:128] = u128
        cb[:, 128:256] = ident
        cb[:, 256:384] = jgrid
        cb[:, 384:384 + NCH] = tvals
        cb[:, 448:449] = dumpo
        cb[:, 449:449 + NH] = g1wi
        cb[:, 481:481 + NH] = b1wi
        cb[0:NCH, 521:521 + NCH] = uN
        cb[0, 585:713] = 1.0
        cb[0, 713:713 + E] = np.asarray(gate_bi, f32).reshape(E)
        cb[0, 721:721 + D] = np.asarray(b2i, f32).reshape(D)
        cb[0, 721 + D:721 + 2 * D] = np.asarray(g2i, f32).reshape(D)
        return cb

    def bf16(a):
        return np.ascontiguousarray(np.asarray(a, f32)).astype(bf)

    def wrap_pc(v, nch):  # [nch*128] -> [128, nch] with v[c*128+p] at [p, c]
        return np.ascontiguousarray(np.asarray(v, f32).reshape(nch, 128).T)

    in_maps = []
    for i in range(cfg.NCORES):
        in_maps.append({
            "xbfp": xbfp,
            "xslice": np.ascontiguousarray(xf[i * TSL:(i + 1) * TSL]),
            "gatev": np.ascontiguousarray(gate_v, dtype=f32),
            "gateg": np.ascontiguousarray(np.asarray(gate_g, f32).reshape(E, 1)),
            "gateb": np.ascontiguousarray(np.asarray(gate_b, f32).reshape(1, E)),
            "w1b": bf16(w1_v[i]),
            "w2b": bf16(w2_v[i]),
            "cblk": pack_cblk(wrap_pc(w1_g[i], NH), wrap_pc(b1[i], NH),
                              w2_g[i], gate_b, b2[i]),
            "ones1b": bf16(ones1),
            "onescb": bf16(np.ones((128, 1), f32)),
        })
    return in_maps


_COMPILED = {}


def compute_routing(cfg: Cfg, x, gate_v, gate_g, gate_b):
    """Host-side routing (f64) -> per-chunk counts for windows/capacity."""
    T, D, E = cfg.T, cfg.D, cfg.E
    NCH = cfg.NCH
    xf = np.asarray(x, np.float64).reshape(T, D)
    gv = np.asarray(gate_v, np.float64)
    gw = gv / np.maximum(np.sqrt((gv * gv).sum(-1, keepdims=True)), 1e-12)
    gw = gw * np.asarray(gate_g, np.float64).reshape(E, 1)
    logits = xf @ gw.T + np.asarray(gate_b, np.float64).reshape(1, E)
    part = np.argpartition(-logits, 2, axis=1)[:, :2]
    cnts = np.zeros((E, NCH), np.int64)
    for e in range(E):
        m = (part == e).any(1)
        cnts[e] = m.reshape(NCH, 128).sum(1)
    return cnts


def pick_capacity(cfg: Cfg, cnts, margin=33):
    maxcnt = int(cnts.sum(1).max())
    C = ((maxcnt + margin + 127) // 128) * 128
    return C


def compute_windows(cfg: Cfg, cnts, margin=32):
    """Conservative (J -> chunk range) windows for the gather-list build."""
    C, NCH, NBJ = cfg.C, cfg.NCH, cfg.C // 128
    lo = [NCH] * NBJ
    hi = [-1] * NBJ
    for e in range(cfg.E):
        cnt = cnts[e]
        offs = np.concatenate([[0], np.cumsum(cnt)[:-1]])
        for g in range(NCH):
            a = max(0, int(offs[g]) - margin)
            b = min(C, int(offs[g] + cnt[g]) + margin)
            if b <= a:
                b = a + 1
            for J in range(a // 128, min(NBJ - 1, (b - 1) // 128) + 1):
                lo[J] = min(lo[J], g)
                hi[J] = max(hi[J], g)
    return tuple((min(lo[J], NCH - 1), max(hi[J], min(lo[J], NCH - 1)))
                 for J in range(NBJ))


def get_compiled(cfg: Cfg, windows=None):
    key = (cfg.T, cfg.D, cfg.H, cfg.E, cfg.C, tuple(cfg.groups), windows)
    if key not in _COMPILED:
        nc = bacc.Bacc("TRN2", target_bir_lowering=False, debug=False,
                       num_devices=cfg.NCORES)
        build_moe(nc, cfg, windows)
        nc.compile()
        _COMPILED[key] = nc
    return _COMPILED[key]


def plan(x, gate_v, gate_g, gate_b):
    cfg0 = Cfg()
    cnts = compute_routing(cfg0, x, gate_v, gate_g, gate_b)
    C = pick_capacity(cfg0, cnts)
    cfg = Cfg(C=C)
    windows = compute_windows(cfg, cnts)
    return cfg, windows


def kernel(x, gate_v, gate_g, gate_b, w1_v, w1_g, b1, w2_v, w2_g, b2):
    from concourse.bass_utils import run_bass_kernel_spmd

    cfg, windows = plan(x, gate_v, gate_g, gate_b)
    nc = get_compiled(cfg, windows)
    in_maps = make_in_maps(cfg, np.asarray(x), np.asarray(gate_v),
                           np.asarray(gate_g), np.asarray(gate_b),
                           np.asarray(w1_v), np.asarray(w1_g), np.asarray(b1),
                           np.asarray(w2_v), np.asarray(w2_g), np.asarray(b2))
    res = run_bass_kernel_spmd(nc, in_maps, core_ids=list(range(cfg.NCORES)))
    shards = [res.results[i]["out"] for i in range(cfg.NCORES)]
    out = unpermute(cfg, np.stack(shards, axis=0))
    B, S_, D_ = x.shape
    return out.reshape(B, S_, D_)


def unpermute(cfg: Cfg, shards):
    """Chunked-RS row order -> token order.

    RS chunk k hands rank i rows [2048k + 256i, 2048k + 256(i+1)) of the
    token axis, stored at rs_out rows [256k, 256(k+1)).
    """
    arr = np.asarray(shards, np.float32)       # [8, TSL, D]
    n, tsl, d = arr.shape
    nrs = cfg.NRS
    blk = tsl // nrs
    return arr.reshape(n, nrs, blk, d).transpose(1, 0, 2, 3).reshape(n * tsl, d)
